# revision 29
# baseline (speedup 1.0000x reference)
"""CPR linear (int8-dequant matmul with column reordering) on 8 Trainium2
NeuronCores.

Math: y = x[:, col_indices] @ (W_int8 * repeat(scales, gs)) + bias
Equivalently, with inv = argsort(col_indices):
    y[m, j-contraction] = sum_j x[m, j] * W[inv[j], n] * scales[inv[j]//gs, n]
so x is consumed in natural column order and the permutation rides on W's
rows (host-side index gather; W is 8x smaller than x).

Sharding: column-parallel. Each core owns 512 output features: its slices
of W (row-permuted), per-row scale rows, and bias; x is replicated.

Per-core device kernel:
  - bias broadcast [512] -> [128, 512] via DMA
  - dequant: wd[k,n] = wbf[k,n] * sbf[k,n] (bf16), resident 4MB in SBUF
  - main loop over 8 m-blocks of 1024 rows:
      32 DMA-transpose loads  x[mb, kt] -> xT [128k, 1024m] bf16
      8 m-subtiles x 32 k-tiles accumulating matmuls into PSUM [128, 512] f32
      PSUM + bias -> SBUF -> DMA out
"""
from contextlib import ExitStack

import numpy as np
import ml_dtypes

import concourse.bass as bass
import concourse.bacc as bacc
import concourse.mybir as mybir
import concourse.tile as tile

B, S, K, N = 4, 2048, 4096, 4096
M = B * S                    # 8192
NCORES = 8
NS = N // NCORES             # 512 output cols per core
P = 128
NKT = K // P                 # 32 k-tiles
MB = 1024                    # m-block rows
NMB = M // MB                # 8
MSUB = MB // P               # 8

bf16 = mybir.dt.bfloat16
f32 = mybir.dt.float32


KB = 4                       # k-tiles batched per x-load DMA (1MB transfers)
NKG = NKT // KB              # 8 k-groups


def build(repeats: int = 1, variant: str = "full"):
    """variant: "full" | "nomm" (DMA/DVE path only) | "mmonly" (PE path only)
    | "mmonly256" (PE path, half-width moving operand)"""
    do_mm = variant in ("full", "mmonly", "mmonly256")
    do_xdma = variant in ("full", "nomm")
    nw = 256 if variant == "mmonly256" else NS

    nc = bacc.Bacc(None)
    # x supplied pre-transposed [K, M] bf16 (host does cast + transpose)
    x_d = nc.dram_tensor("xbf", [K, M], bf16, kind="ExternalInput")
    w_d = nc.dram_tensor("wbf", [K, NS], bf16, kind="ExternalInput")
    s_d = nc.dram_tensor("sbf", [K, NS], bf16, kind="ExternalInput")
    b_d = nc.dram_tensor("bias", [NS], f32, kind="ExternalInput")
    y_d = nc.dram_tensor("y", [M, NS], f32, kind="ExternalOutput")

    with tile.TileContext(nc) as tc, ExitStack() as stk:
        if repeats > 1:
            stk.enter_context(tc.For_i(0, repeats, 1))
        with (
            tc.tile_pool(name="consts", bufs=1) as consts,
            tc.tile_pool(name="xpool", bufs=2) as xpool,
            tc.tile_pool(name="opool", bufs=2) as opool,
            tc.tile_pool(name="psum", bufs=4, space="PSUM") as psum_pool,
        ):
            # dequantized weights, resident: [128, NKT*NS] bf16 (4MB).
            # W and scale rows staged in chunks, smallest first, so the first
            # matmuls gate on only a 0.25MB load + one small dequant.
            bias_t = consts.tile([P, NS], f32)
            wd = consts.tile([P, NKT * NS], bf16)
            with tc.tile_pool(name="wstage", bufs=2) as wstage:
                W_CHUNKS = [2, 6, 8, 8, 8]
                k0 = 0
                for h, H in enumerate(W_CHUNKS):
                    r = slice(k0 * P, (k0 + H) * P)
                    wraw = wstage.tile([P, 8, NS], bf16, tag="wraw")
                    nc.scalar.dma_start(
                        out=wraw[:, :H],
                        in_=w_d[r, :].rearrange("(t p) n -> p t n", p=P))
                    sraw = wstage.tile([P, 8, NS], bf16, tag="sraw")
                    nc.scalar.dma_start(
                        out=sraw[:, :H],
                        in_=s_d[r, :].rearrange("(t p) n -> p t n", p=P))
                    nc.vector.tensor_tensor(
                        out=wd[:, k0 * NS:(k0 + H) * NS],
                        in0=wraw[:, :H].opt(), in1=sraw[:, :H].opt(),
                        op=mybir.AluOpType.mult,
                    )
                    k0 += H

            # bias broadcast to all partitions (needed only at first PSUM
            # eviction, so issued after the W loads on the same queue)
            nc.scalar.dma_start(
                out=bias_t,
                in_=bass.AP(tensor=b_d, offset=0, ap=[[0, P], [1, NS]]),
            )

            xT_static = None
            if not do_xdma:
                xT_static = []
                for kg in range(NKG):
                    ts_tile = consts.tile([P, KB, MB], bf16, tag=f"xTs{kg}")
                    nc.vector.memset(ts_tile, 0.5)
                    xT_static.append(ts_tile)

            for mb in range(NMB):
                m0 = mb * MB
                if do_xdma:
                    xT = []
                    for kg in range(NKG):
                        t = xpool.tile([P, KB, MB], bf16, tag=f"xT{kg}")
                        src = x_d[kg * KB * P:(kg + 1) * KB * P, m0:m0 + MB]
                        nc.sync.dma_start(
                            out=t, in_=src.rearrange("(b p) m -> p b m", p=P),
                        )
                        xT.append(t)
                else:
                    xT = xT_static
                if not do_mm:
                    continue
                # process m-subtiles in pairs: two PSUM banks accumulate,
                # both evict into one [128, 2, nw] tile, one 512KB store
                for msp in range(MSUB // 2):
                    ot = opool.tile([P, 2, nw], f32, tag="ot")
                    for half in range(2):
                        ms = msp * 2 + half
                        ps = psum_pool.tile([P, nw], f32, tag="ps")
                        for kt in range(NKT):
                            nc.tensor.matmul(
                                ps,
                                xT[kt // KB][:, kt % KB, ms * P:(ms + 1) * P],
                                wd[:, kt * NS:kt * NS + nw],
                                start=(kt == 0), stop=(kt == NKT - 1),
                            )
                        nc.vector.tensor_tensor(
                            out=ot[:, half], in0=ps, in1=bias_t[:, :nw],
                            op=mybir.AluOpType.add,
                        )
                    row0 = m0 + msp * 2 * P
                    dst = y_d[row0:row0 + 2 * P, :nw]
                    nc.scalar.dma_start(
                        out=dst.rearrange("(b p) n -> p b n", p=P), in_=ot,
                    )

    nc.compile()
    return nc


def make_in_maps(x, scales, bias, weight_int8, col_indices, group_size):
    """Host-side sharding/layout prep: index gathers and dtype casts only."""
    gs = int(group_size)
    x2 = np.asarray(x, dtype=np.float32).reshape(M, K)
    x_bf = x2.T.astype(ml_dtypes.bfloat16, order="C")   # [K, M], bf16

    ci = np.asarray(col_indices).astype(np.int64)
    inv = np.argsort(ci)                     # inv[j]: W row paired with x col j
    gi = inv // gs                           # scale group per permuted row

    Wp = np.asarray(weight_int8)[inv]        # [K, N], int32 values in [-128,127]
    sc = np.asarray(scales, dtype=np.float32)
    bias = np.asarray(bias, dtype=np.float32)

    in_maps = []
    for c in range(NCORES):
        cols = slice(c * NS, (c + 1) * NS)
        in_maps.append({
            "xbf": x_bf,
            "wbf": Wp[:, cols].astype(ml_dtypes.bfloat16),   # exact (ints)
            "sbf": sc[:, cols][gi].astype(ml_dtypes.bfloat16),
            "bias": bias[cols],
        })
    return in_maps


_RUNNER = None


def _make_runner():
    """Build the bass module once and wrap it in a cached sharded jit."""
    import jax
    from jax.sharding import Mesh, PartitionSpec, NamedSharding
    from jax.experimental.shard_map import shard_map
    from concourse import bass2jax
    from concourse.bass2jax import _bass_exec_p, install_neuronx_cc_hook

    nc = build(repeats=1)
    install_neuronx_cc_hook()
    partition_name = nc.partition_id_tensor.name if nc.partition_id_tensor else None

    in_names, out_names, out_avals, zero_outs = [], [], [], []
    for alloc in nc.m.functions[0].allocations:
        if not isinstance(alloc, mybir.MemoryLocationSet):
            continue
        name = alloc.memorylocations[0].name
        if alloc.kind == "ExternalInput":
            if name != partition_name:
                in_names.append(name)
        elif alloc.kind == "ExternalOutput":
            out_names.append(name)
            shape = tuple(alloc.tensor_shape)
            dtype = mybir.dt.np(alloc.dtype)
            out_avals.append(jax.core.ShapedArray(shape, dtype))
            zero_outs.append(np.zeros(shape, dtype))
    all_in_names = list(in_names) + list(out_names)
    if partition_name is not None:
        all_in_names.append(partition_name)
    n_params, n_outs = len(in_names), len(out_names)

    def _body(*args):
        operands = list(args)
        if partition_name is not None:
            operands.append(bass2jax.partition_id_tensor())
        outs = _bass_exec_p.bind(
            *operands,
            out_avals=tuple(out_avals),
            in_names=tuple(all_in_names),
            out_names=tuple(out_names),
            lowering_input_output_aliases=(),
            sim_require_finite=True,
            sim_require_nnan=True,
            nc=nc,
        )
        return tuple(outs)

    devices = jax.devices()[:NCORES]
    mesh = Mesh(np.asarray(devices), ("core",))
    # x ("xbf") is identical on every core: pass it replicated so only one
    # copy crosses the host->device link; per-core tensors are concat-sharded.
    in_specs = tuple(
        PartitionSpec() if name == "xbf" else PartitionSpec("core")
        for name in in_names
    ) + (PartitionSpec("core"),) * n_outs
    sharded = jax.jit(
        shard_map(
            _body, mesh=mesh,
            in_specs=in_specs,
            out_specs=(PartitionSpec("core"),) * n_outs,
            check_rep=False,
        ),
        keep_unused=True,
    )
    shard_core = NamedSharding(mesh, PartitionSpec("core"))
    shard_repl = NamedSharding(mesh, PartitionSpec())

    def run(in_maps):
        import jax as _jax
        dev_in = []
        for name in in_names:
            if name == "xbf":
                dev_in.append(
                    _jax.device_put(np.asarray(in_maps[0][name]), shard_repl))
            else:
                a = np.concatenate(
                    [np.asarray(in_maps[c][name]) for c in range(NCORES)], axis=0)
                dev_in.append(_jax.device_put(a, shard_core))
        dev_zero = [
            _jax.device_put(
                np.zeros((NCORES * z.shape[0], *z.shape[1:]), z.dtype), shard_core)
            for z in zero_outs
        ]
        out = sharded(*dev_in, *dev_zero)
        return [
            {name: np.asarray(out[i]).reshape(NCORES, *zero_outs[i].shape)[c]
             for i, name in enumerate(out_names)}
            for c in range(NCORES)
        ]

    return run


def kernel(x, scales, bias, weight_int8, col_indices, group_size):
    global _RUNNER
    in_maps = make_in_maps(x, scales, bias, weight_int8, col_indices, group_size)
    if _RUNNER is None:
        _RUNNER = _make_runner()
    results = _RUNNER(in_maps)
    y = np.concatenate([results[c]["y"] for c in range(NCORES)], axis=1)
    return np.ascontiguousarray(y.reshape(B, S, N))
